# revision 33
# baseline (speedup 1.0000x reference)
"""MoE (top-2 of 8 experts) Trainium2 kernel.

Strategy: expert-parallel across the 8 NeuronCores. The router (a tiny
[T,512]@[512,8] matmul + softmax + top-k, ~0.02% of the layer's FLOPs) runs
on host bit-identically to the reference (jax on CPU). Tokens are gathered
per expert on host, padded to a common capacity C, and each core computes
its expert's full FFN on device:

    outT = (w2.T @ gelu(w1.T @ xT + b1) + b2) * gate

in a transposed layout (features on partitions, tokens on the moving/free
axis) so both matmuls chain on the TensorEngine with no transposes, and the
b1/b2 biases are free per-partition operands. The gate multiply uses a
partition-broadcast gate row. Host scatter-adds the two expert
contributions per token back into the full [B,S,D] output.

Only the selected top-2 experts contribute to the reference output (the
gate is exactly zero elsewhere), so this computes 4x fewer FLOPs than the
dense reference while being numerically equivalent.

All device inputs are packed on host into contiguous ~1MB blocks laid out
in exactly the order the kernel consumes them: HWDGE drains the sync ring
FIFO, so consumption-ordered contiguous blocks give both full DMA
bandwidth and earliest possible compute start.
"""

import os
import sys

sys.path.insert(0, "/opt/trn_rl_repo")

import numpy as np

TOP_K = 2
N_CORES = 8
P = 128  # SBUF partitions

# Matmul dtype: "float32" (exact, 4 cyc/row) or "float32r" (1 cyc/row at
# N>=256, TF32-like internal precision, ~2e-4 rel err end to end).
MM_DT = os.environ.get("MOE_MM_DT", "float32r")
NTILE = 512  # moving-operand (token) tile; max for 4-byte dtypes
MG = 512  # w1 column-block (4 m-tiles per block)
ACT_FUNC = os.environ.get("MOE_ACT_FUNC", "Gelu")  # CoreSim lacks Gelu; Tanh for sim


def _route(x_flat, gate_w, gate_b):
    """Reference router, bit-identical: jax on CPU."""
    import jax
    import jax.numpy as jnp

    with jax.default_device(jax.devices("cpu")[0]):
        logits = jnp.asarray(x_flat) @ jnp.asarray(gate_w) + jnp.asarray(gate_b)
        raw_weights = jax.nn.softmax(logits, axis=-1)
        top_w, top_idx = jax.lax.top_k(raw_weights, TOP_K)
        return np.asarray(top_w), np.asarray(top_idx)


def _tile_sizes(C):
    return [min(NTILE, C - c0) for c0 in range(0, C, NTILE)]


def _pack_inputs(XT, G, w1e, b1e, w2e, b2e, C, D, H):
    """Pack one expert's inputs into the kernel's blocked layouts."""
    KT, MT, DT = D // P, H // P, D // P
    MGn, MTG = H // MG, MT // 4
    xt_blocks = []
    for i, csz in enumerate(_tile_sizes(C)):
        c0 = i * NTILE
        xt_blocks.append(
            XT.reshape(KT, P, C)[:, :, c0 : c0 + csz].transpose(1, 0, 2).ravel()
        )
    return {
        "xt": np.ascontiguousarray(np.concatenate(xt_blocks)),
        "g": np.ascontiguousarray(G.reshape(1, C)),
        "w1": np.ascontiguousarray(
            w1e.reshape(KT, P, MT, P).transpose(2, 1, 0, 3)
        ),
        "b1": np.ascontiguousarray(b1e.reshape(MT, P).T),
        "w2": np.ascontiguousarray(
            w2e.reshape(MTG, 4, P, D).transpose(0, 2, 1, 3)
        ),
        "b2": np.ascontiguousarray(b2e.reshape(DT, P).T),
    }


def _unpack_out(flat, C, D):
    """Blocked per-(n,d) output -> outT [D, C]."""
    DT = D // P
    outT = np.empty((D, C), np.float32)
    off = 0
    for i, csz in enumerate(_tile_sizes(C)):
        c0 = i * NTILE
        for d in range(DT):
            outT[d * P : (d + 1) * P, c0 : c0 + csz] = flat[
                off : off + P * csz
            ].reshape(P, csz)
            off += P * csz
    return outT


def _build_program(C, D, H, mm_dt_name):
    """Build the per-core Bass program (identical on all cores)."""
    import concourse.bass as bass
    import concourse.mybir as mybir
    import concourse.tile as tile
    from concourse import bacc
    from concourse.tile_rust import add_dep_helper

    f32 = mybir.dt.float32
    mm_dt = getattr(mybir.dt, mm_dt_name)
    act = getattr(mybir.ActivationFunctionType, ACT_FUNC)
    KT = D // P  # 4  k-tiles for matmul1 (contraction over D)
    MT = H // P  # 16 m-tiles (H rows of hT)
    DT = D // P  # 4  d-tiles of the output
    MGn = H // MG  # 4  w1 column blocks
    MTG = MT // 4  # 4  w2 row-block groups
    sizes = _tile_sizes(C)
    NT = len(sizes)

    nc = bacc.Bacc(None, target_bir_lowering=False, debug=False)
    xt_h = nc.dram_tensor("xt", [P * KT * C], mm_dt, kind="ExternalInput")
    g_h = nc.dram_tensor("g", [1, C], f32, kind="ExternalInput")
    w1_h = nc.dram_tensor("w1", [MT, P, KT, P], mm_dt, kind="ExternalInput")
    b1_h = nc.dram_tensor("b1", [P, MT], f32, kind="ExternalInput")
    w2_h = nc.dram_tensor("w2", [MTG, P, 4, D], mm_dt, kind="ExternalInput")
    b2_h = nc.dram_tensor("b2", [P, DT], f32, kind="ExternalInput")
    out_h = nc.dram_tensor("out", [P * DT * C], f32, kind="ExternalOutput")

    with tile.TileContext(nc) as tc:
        with (
            tc.tile_pool(name="weights", bufs=1) as wpool,
            tc.tile_pool(name="xio", bufs=2) as xio,
            tc.tile_pool(name="gio", bufs=2) as gio,
            tc.tile_pool(name="oio", bufs=3) as oio,
            tc.tile_pool(name="hbuf", bufs=1) as hbuf,
            tc.tile_pool(name="ps1", bufs=4, space=bass.MemorySpace.PSUM) as ps1,
            # matmul2 keeps DT banks live across its whole m-loop; bufs=1
            # per d-tag (release happens at the DVE evacuation, early in
            # the next n-tile's matmul1 phase). 4 + 4 = 8 banks.
            tc.tile_pool(name="ps2", bufs=1, space=bass.MemorySpace.PSUM) as ps2,
        ):
            # DMA issue order == consumption order (sync ring is FIFO):
            # xt[n0], w1 blocks, then w2 blocks / biases, then per-n IO.
            # One tile per weight block — Tile deps are per-tile, so a
            # single multi-DMA tile would stall the first matmul on the
            # LAST block's DMA.
            # Sync-ring stream order (the sequencer head-of-line-blocks at
            # each gated DMA, serializing everything behind it — by
            # design): tiny g row, xt0, w1_0 and b1 land immediately; the
            # remaining w1 blocks stream just-in-time behind matmul gates;
            # w2/b2/g-broadcast follow once n0's matmul1 is underway.
            xt_tiles = {}
            xt_tiles[0] = xio.tile([P, KT, sizes[0]], mm_dt, tag="xt", name="xt0")
            nc.sync.dma_start(
                out=xt_tiles[0],
                in_=xt_h.ap()[0 : P * KT * sizes[0]].rearrange(
                    "(p kt c) -> p kt c", p=P, kt=KT
                ),
            )
            w1_t = [wpool.tile([P, KT, P], mm_dt, name=f"w1_{m}") for m in range(MT)]
            w1_dmas = [nc.sync.dma_start(out=w1_t[m], in_=w1_h.ap()[m]) for m in range(4)]
            b1_sb = wpool.tile([P, MT], f32)
            nc.sync.dma_start(out=b1_sb, in_=b1_h.ap())
            for m in range(4, MT):
                w1_dmas.append(nc.sync.dma_start(out=w1_t[m], in_=w1_h.ap()[m]))
            b2_sb = wpool.tile([P, DT], f32)
            b2_dma = nc.sync.dma_start(out=b2_sb, in_=b2_h.ap())
            w2_t = []
            w2_dmas = [b2_dma]
            for mtg in range(MTG):
                t = wpool.tile([P, 4, D], mm_dt, name=f"w2_{mtg}")
                w2_dmas.append(nc.sync.dma_start(out=t, in_=w2_h.ap()[mtg]))
                w2_t.append(t)
            # broadcast the gate row across partitions in one HWDGE DMA
            # (reads the 9KB row 128x from HBM; no SWDGE descriptor traffic)
            g_full = gio.tile([P, C], f32, name="g_full")
            nc.sync.dma_start(out=g_full, in_=g_h.ap().partition_broadcast(P))

            xt_off = P * KT * sizes[0]
            out_off = 0
            # DMA-priority gating: everything not needed for the first
            # m-tiles is held back behind early n0 compute, so the ring
            # round-robin doesn't starve the critical xt0+w1 stream.
            gate_act = None  # gelu[n0, m=6]: releases w2 blocks
            prev_first_act = None  # gelu[n-1, m=0]: releases n's xt/g DMAs
            for n in range(NT):
                csz = sizes[n]
                c0 = n * NTILE
                if n in xt_tiles:
                    xt_t = xt_tiles.pop(n)
                else:
                    xt_t = xio.tile([P, KT, csz], mm_dt, tag="xt", name="xt")
                    dma = nc.sync.dma_start(
                        out=xt_t,
                        in_=xt_h.ap()[xt_off : xt_off + P * KT * csz].rearrange(
                            "(p kt c) -> p kt c", p=P, kt=KT
                        ),
                    )
                    if prev_first_act is not None:
                        add_dep_helper(dma.ins, prev_first_act.ins, reason="stagger xt load")
                    xt_off += P * KT * csz
                g_t = g_full[:, c0 : c0 + csz]
                hT = hbuf.tile([P, MT, csz], mm_dt, tag="hT", name="hT")
                first_act = None
                for m in range(MT):
                    pst = ps1.tile([P, csz], f32, tag="ps1", name="ps1")
                    for kt in range(KT):
                        mm = nc.tensor.matmul(
                            pst,
                            lhsT=w1_t[m][:, kt, :],
                            rhs=xt_t[:, kt, :],
                            start=(kt == 0),
                            stop=(kt == KT - 1),
                        )
                        # just-in-time w1 streaming with one group of
                        # slack: blocks m+4..m+7 released by the first
                        # matmul that consumes block m (m0-3 ungated)
                        if n == 0 and kt == 0 and m % 4 == 0 and m + 4 < MT:
                            for j in range(m + 4, min(m + 8, MT)):
                                add_dep_helper(
                                    w1_dmas[j].ins,
                                    mm.ins,
                                    reason="stagger w1 load",
                                )
                    a = nc.scalar.activation(
                        out=hT[:, m, :],
                        in_=pst,
                        func=act,
                        bias=b1_sb[:, m : m + 1],
                        scale=1.0,
                    )
                    if m == 0:
                        first_act = a
                    if n == 0 and m == 6:
                        gate_act = a
                        for dma in w2_dmas:
                            add_dep_helper(dma.ins, gate_act.ins, reason="stagger w2 load")
                prev_first_act = first_act
                # matmul2 with m as the OUTER loop: w2 blocks are consumed
                # in DMA-arrival order, so the first n-tile never stalls on
                # the tail of the weight stream. Needs DT live PSUM banks.
                # The LAST tile uses d-outer instead so each d's evacuation
                # and store overlaps the remaining matmuls (shorter tail).
                def evac(pso_d, d, ot):
                    nc.vector.scalar_tensor_tensor(
                        out=ot[:, d, :],
                        in0=pso_d,
                        scalar=b2_sb[:, d : d + 1],
                        in1=g_t,
                        op0=mybir.AluOpType.add,
                        op1=mybir.AluOpType.mult,
                    )

                def store(ot, d0, nd):
                    # dram block order is [d][p][c]; SBUF is [p][d][c]
                    nonlocal out_off
                    nc.sync.dma_start(
                        out=out_h.ap()[out_off : out_off + nd * P * csz].rearrange(
                            "(dt p c) -> p dt c", p=P, dt=nd
                        ),
                        in_=ot[:, d0 : d0 + nd, :],
                    )
                    out_off += nd * P * csz

                ot = oio.tile([P, DT, csz], f32, tag="ot", name="ot")
                if n < NT - 1:
                    pso = [
                        ps2.tile([P, csz], f32, tag=f"ps2_{d}", name=f"ps2_{d}")
                        for d in range(DT)
                    ]
                    for m in range(MT):
                        for d in range(DT):
                            nc.tensor.matmul(
                                pso[d],
                                lhsT=w2_t[m // 4][:, m % 4, d * P : (d + 1) * P],
                                rhs=hT[:, m, :],
                                start=(m == 0),
                                stop=(m == MT - 1),
                            )
                    for d in range(DT):
                        evac(pso[d], d, ot)
                    store(ot, 0, DT)  # one trigger per n-tile
                else:
                    # last tile: d-outer so each d's evacuation + store
                    # overlaps the remaining matmuls (shorter tail)
                    for d in range(DT):
                        pso_d = ps2.tile(
                            [P, csz], f32, tag=f"ps2_{d}", name=f"ps2_{d}"
                        )
                        for m in range(MT):
                            nc.tensor.matmul(
                                pso_d,
                                lhsT=w2_t[m // 4][:, m % 4, d * P : (d + 1) * P],
                                rhs=hT[:, m, :],
                                start=(m == 0),
                                stop=(m == MT - 1),
                            )
                        evac(pso_d, d, ot)
                        store(ot, d, 1)

    nc.compile()
    return nc


def _run(nc, in_maps, trace=False):
    from concourse.bass_utils import run_bass_kernel_spmd

    if trace:
        # register the NTFF profiling hook (missing antenv.axon_hooks shim)
        import types

        import antenv

        if not hasattr(antenv, "axon_hooks"):
            mod = types.ModuleType("antenv.axon_hooks")
            _hook = [None]
            mod.set_axon_ntff_profile_hook = lambda h: _hook.__setitem__(0, h)
            mod.get_axon_ntff_profile_hook = lambda: _hook[0]
            sys.modules["antenv.axon_hooks"] = mod
            antenv.axon_hooks = mod
            from trn_agent_boot.trn_boot import _ntff_profile_via_ctypes

            mod.set_axon_ntff_profile_hook(
                _ntff_profile_via_ctypes("/opt/axon/libaxon_pjrt.so")
            )
    return run_bass_kernel_spmd(
        nc, in_maps, core_ids=list(range(N_CORES)), trace=trace
    )


def kernel(x, gate_w, gate_b, w1, b1, w2, b2, _trace=False):
    x = np.ascontiguousarray(np.asarray(x, dtype=np.float32))
    gate_w = np.asarray(gate_w, dtype=np.float32)
    gate_b = np.asarray(gate_b, dtype=np.float32)
    w1 = np.asarray(w1, dtype=np.float32)
    b1 = np.asarray(b1, dtype=np.float32)
    w2 = np.asarray(w2, dtype=np.float32)
    b2 = np.asarray(b2, dtype=np.float32)

    B, S, D = x.shape
    E = gate_w.shape[1]
    H = w1.shape[2]
    assert E == N_CORES
    T = B * S
    x_flat = x.reshape(T, D)

    top_w, top_idx = _route(x_flat, gate_w, gate_b)

    toks, gvals = [], []
    for e in range(E):
        mask = top_idx == e  # [T, K]; at most one True per row
        t_ids = np.nonzero(mask.any(axis=1))[0]
        toks.append(t_ids)
        gvals.append(top_w[mask].astype(np.float32))
    Cmax = max(len(t) for t in toks)
    C = max(((Cmax + P - 1) // P) * P, NTILE)

    in_maps = []
    for e in range(E):
        cnt = len(toks[e])
        XT = np.zeros((D, C), np.float32)
        XT[:, :cnt] = x_flat[toks[e]].T
        G = np.zeros((1, C), np.float32)
        G[0, :cnt] = gvals[e]
        in_maps.append(_pack_inputs(XT, G, w1[e], b1[e], w2[e], b2[e], C, D, H))

    nc = _build_program(C, D, H, MM_DT)
    res = _run(nc, in_maps, trace=_trace)
    global _LAST_RES
    _LAST_RES = res

    out_flat = np.zeros((T, D), np.float32)
    for e in range(E):
        cnt = len(toks[e])
        outT = _unpack_out(res.results[e]["out"], C, D)
        out_flat[toks[e]] += outT[:, :cnt].T

    out = out_flat.reshape(B, S, D)
    if _trace:
        return out, res.exec_time_ns
    return out


# revision 34
# speedup vs baseline: 1.0149x; 1.0149x over previous
"""MoE (top-2 of 8 experts) Trainium2 kernel.

Strategy: expert-parallel across the 8 NeuronCores. The router (a tiny
[T,512]@[512,8] matmul + softmax + top-k, ~0.02% of the layer's FLOPs) runs
on host bit-identically to the reference (jax on CPU). Tokens are gathered
per expert on host, padded to a common capacity C, and each core computes
its expert's full FFN on device:

    outT = (w2.T @ gelu(w1.T @ xT + b1) + b2) * gate

in a transposed layout (features on partitions, tokens on the moving/free
axis) so both matmuls chain on the TensorEngine with no transposes, and the
b1/b2 biases are free per-partition operands. The gate multiply uses a
partition-broadcast gate row. Host scatter-adds the two expert
contributions per token back into the full [B,S,D] output.

Only the selected top-2 experts contribute to the reference output (the
gate is exactly zero elsewhere), so this computes 4x fewer FLOPs than the
dense reference while being numerically equivalent.

All device inputs are packed on host into contiguous ~1MB blocks laid out
in exactly the order the kernel consumes them: HWDGE drains the sync ring
FIFO, so consumption-ordered contiguous blocks give both full DMA
bandwidth and earliest possible compute start.
"""

import os
import sys

sys.path.insert(0, "/opt/trn_rl_repo")

import numpy as np

TOP_K = 2
N_CORES = 8
P = 128  # SBUF partitions

# Matmul dtype: "float32" (exact, 4 cyc/row) or "float32r" (1 cyc/row at
# N>=256, TF32-like internal precision, ~2e-4 rel err end to end).
MM_DT = os.environ.get("MOE_MM_DT", "float32r")
NTILE = 512  # moving-operand (token) tile; max for 4-byte dtypes
MG = 512  # w1 column-block (4 m-tiles per block)
ACT_FUNC = os.environ.get("MOE_ACT_FUNC", "Gelu")  # CoreSim lacks Gelu; Tanh for sim


def _route(x_flat, gate_w, gate_b):
    """Reference router, bit-identical: jax on CPU."""
    import jax
    import jax.numpy as jnp

    with jax.default_device(jax.devices("cpu")[0]):
        logits = jnp.asarray(x_flat) @ jnp.asarray(gate_w) + jnp.asarray(gate_b)
        raw_weights = jax.nn.softmax(logits, axis=-1)
        top_w, top_idx = jax.lax.top_k(raw_weights, TOP_K)
        return np.asarray(top_w), np.asarray(top_idx)


def _tile_sizes(C):
    return [min(NTILE, C - c0) for c0 in range(0, C, NTILE)]


def _pack_inputs(XT, G, w1e, b1e, w2e, b2e, C, D, H):
    """Pack one expert's inputs into the kernel's blocked layouts."""
    KT, MT, DT = D // P, H // P, D // P
    MGn, MTG = H // MG, MT // 4
    xt_blocks = []
    for i, csz in enumerate(_tile_sizes(C)):
        c0 = i * NTILE
        xt_blocks.append(
            XT.reshape(KT, P, C)[:, :, c0 : c0 + csz].transpose(1, 0, 2).ravel()
        )
    return {
        "xt": np.ascontiguousarray(np.concatenate(xt_blocks)),
        "g": np.ascontiguousarray(G.reshape(1, C)),
        "w1": np.ascontiguousarray(
            w1e.reshape(KT, P, MT, P).transpose(2, 1, 0, 3)
        ),
        "b1": np.ascontiguousarray(b1e.reshape(MT, P).T),
        "w2": np.ascontiguousarray(
            w2e.reshape(MTG, 4, P, D).transpose(0, 2, 1, 3)
        ),
        "b2": np.ascontiguousarray(b2e.reshape(DT, P).T),
    }


def _unpack_out(flat, C, D):
    """Blocked per-(n,d) output -> outT [D, C]."""
    DT = D // P
    outT = np.empty((D, C), np.float32)
    off = 0
    for i, csz in enumerate(_tile_sizes(C)):
        c0 = i * NTILE
        for d in range(DT):
            outT[d * P : (d + 1) * P, c0 : c0 + csz] = flat[
                off : off + P * csz
            ].reshape(P, csz)
            off += P * csz
    return outT


def _build_program(C, D, H, mm_dt_name):
    """Build the per-core Bass program (identical on all cores)."""
    import concourse.bass as bass
    import concourse.mybir as mybir
    import concourse.tile as tile
    from concourse import bacc
    from concourse.tile_rust import add_dep_helper

    f32 = mybir.dt.float32
    mm_dt = getattr(mybir.dt, mm_dt_name)
    act = getattr(mybir.ActivationFunctionType, ACT_FUNC)
    KT = D // P  # 4  k-tiles for matmul1 (contraction over D)
    MT = H // P  # 16 m-tiles (H rows of hT)
    DT = D // P  # 4  d-tiles of the output
    MGn = H // MG  # 4  w1 column blocks
    MTG = MT // 4  # 4  w2 row-block groups
    sizes = _tile_sizes(C)
    NT = len(sizes)

    nc = bacc.Bacc(None, target_bir_lowering=False, debug=False)
    xt_h = nc.dram_tensor("xt", [P * KT * C], mm_dt, kind="ExternalInput")
    g_h = nc.dram_tensor("g", [1, C], f32, kind="ExternalInput")
    w1_h = nc.dram_tensor("w1", [MT, P, KT, P], mm_dt, kind="ExternalInput")
    b1_h = nc.dram_tensor("b1", [P, MT], f32, kind="ExternalInput")
    w2_h = nc.dram_tensor("w2", [MTG, P, 4, D], mm_dt, kind="ExternalInput")
    b2_h = nc.dram_tensor("b2", [P, DT], f32, kind="ExternalInput")
    out_h = nc.dram_tensor("out", [P * DT * C], f32, kind="ExternalOutput")

    with tile.TileContext(nc) as tc:
        with (
            tc.tile_pool(name="weights", bufs=1) as wpool,
            tc.tile_pool(name="xio", bufs=2) as xio,
            tc.tile_pool(name="gio", bufs=2) as gio,
            tc.tile_pool(name="oio", bufs=3) as oio,
            tc.tile_pool(name="hbuf", bufs=1) as hbuf,
            tc.tile_pool(name="ps1", bufs=4, space=bass.MemorySpace.PSUM) as ps1,
            # matmul2 keeps DT banks live across its whole m-loop; bufs=1
            # per d-tag (release happens at the DVE evacuation, early in
            # the next n-tile's matmul1 phase). 4 + 4 = 8 banks.
            tc.tile_pool(name="ps2", bufs=1, space=bass.MemorySpace.PSUM) as ps2,
        ):
            # DMA issue order == consumption order (sync ring is FIFO):
            # xt[n0], w1 blocks, then w2 blocks / biases, then per-n IO.
            # One tile per weight block — Tile deps are per-tile, so a
            # single multi-DMA tile would stall the first matmul on the
            # LAST block's DMA.
            # Sync-ring stream order (the sequencer head-of-line-blocks at
            # each gated DMA, serializing everything behind it — by
            # design): tiny g row, xt0, w1_0 and b1 land immediately; the
            # remaining w1 blocks stream just-in-time behind matmul gates;
            # w2/b2/g-broadcast follow once n0's matmul1 is underway.
            xt_tiles = {}
            xt_tiles[0] = xio.tile([P, KT, sizes[0]], mm_dt, tag="xt", name="xt0")
            nc.sync.dma_start(
                out=xt_tiles[0],
                in_=xt_h.ap()[0 : P * KT * sizes[0]].rearrange(
                    "(p kt c) -> p kt c", p=P, kt=KT
                ),
            )
            w1_t = [wpool.tile([P, KT, P], mm_dt, name=f"w1_{m}") for m in range(MT)]
            w1_dmas = [nc.sync.dma_start(out=w1_t[m], in_=w1_h.ap()[m]) for m in range(4)]
            b1_sb = wpool.tile([P, MT], f32)
            nc.sync.dma_start(out=b1_sb, in_=b1_h.ap())
            for m in range(4, MT):
                w1_dmas.append(nc.sync.dma_start(out=w1_t[m], in_=w1_h.ap()[m]))
            b2_sb = wpool.tile([P, DT], f32)
            b2_dma = nc.sync.dma_start(out=b2_sb, in_=b2_h.ap())
            # broadcast the gate row across partitions in one HWDGE DMA
            # (reads the 9KB row 128x from HBM; no SWDGE descriptor traffic)
            g_full = gio.tile([P, C], f32, name="g_full")
            nc.sync.dma_start(out=g_full, in_=g_h.ap().partition_broadcast(P))
            w2_t = []
            w2_dmas = [b2_dma]
            for mtg in range(MTG):
                t = wpool.tile([P, 4, D], mm_dt, name=f"w2_{mtg}")
                w2_dmas.append(nc.sync.dma_start(out=t, in_=w2_h.ap()[mtg]))
                w2_t.append(t)

            xt_off = P * KT * sizes[0]
            out_off = 0
            # DMA-priority gating: everything not needed for the first
            # m-tiles is held back behind early n0 compute, so the ring
            # round-robin doesn't starve the critical xt0+w1 stream.
            gate_act = None  # gelu[n0, m=6]: releases w2 blocks
            prev_first_act = None  # gelu[n-1, m=0]: releases n's xt/g DMAs
            for n in range(NT):
                csz = sizes[n]
                c0 = n * NTILE
                if n in xt_tiles:
                    xt_t = xt_tiles.pop(n)
                else:
                    xt_t = xio.tile([P, KT, csz], mm_dt, tag="xt", name="xt")
                    dma = nc.sync.dma_start(
                        out=xt_t,
                        in_=xt_h.ap()[xt_off : xt_off + P * KT * csz].rearrange(
                            "(p kt c) -> p kt c", p=P, kt=KT
                        ),
                    )
                    if prev_first_act is not None:
                        add_dep_helper(dma.ins, prev_first_act.ins, reason="stagger xt load")
                    xt_off += P * KT * csz
                g_t = g_full[:, c0 : c0 + csz]
                hT = hbuf.tile([P, MT, csz], mm_dt, tag="hT", name="hT")
                first_act = None
                for m in range(MT):
                    pst = ps1.tile([P, csz], f32, tag="ps1", name="ps1")
                    for kt in range(KT):
                        mm = nc.tensor.matmul(
                            pst,
                            lhsT=w1_t[m][:, kt, :],
                            rhs=xt_t[:, kt, :],
                            start=(kt == 0),
                            stop=(kt == KT - 1),
                        )
                        # just-in-time w1 streaming with one group of
                        # slack: blocks m+4..m+7 released by the first
                        # matmul that consumes block m (m0-3 ungated)
                        if n == 0 and kt == 0 and m % 4 == 0 and m + 4 < MT:
                            for j in range(m + 4, min(m + 8, MT)):
                                add_dep_helper(
                                    w1_dmas[j].ins,
                                    mm.ins,
                                    reason="stagger w1 load",
                                )
                    a = nc.scalar.activation(
                        out=hT[:, m, :],
                        in_=pst,
                        func=act,
                        bias=b1_sb[:, m : m + 1],
                        scale=1.0,
                    )
                    if m == 0:
                        first_act = a
                    if n == 0 and m == 2:
                        gate_act = a
                        for dma in w2_dmas:
                            add_dep_helper(dma.ins, gate_act.ins, reason="stagger w2 load")
                prev_first_act = first_act
                # matmul2 with m as the OUTER loop: w2 blocks are consumed
                # in DMA-arrival order, so the first n-tile never stalls on
                # the tail of the weight stream. Needs DT live PSUM banks.
                # The LAST tile uses d-outer instead so each d's evacuation
                # and store overlaps the remaining matmuls (shorter tail).
                def evac(pso_d, d, ot):
                    nc.vector.scalar_tensor_tensor(
                        out=ot[:, d, :],
                        in0=pso_d,
                        scalar=b2_sb[:, d : d + 1],
                        in1=g_t,
                        op0=mybir.AluOpType.add,
                        op1=mybir.AluOpType.mult,
                    )

                def store(ot, d0, nd):
                    # dram block order is [d][p][c]; SBUF is [p][d][c]
                    nonlocal out_off
                    nc.sync.dma_start(
                        out=out_h.ap()[out_off : out_off + nd * P * csz].rearrange(
                            "(dt p c) -> p dt c", p=P, dt=nd
                        ),
                        in_=ot[:, d0 : d0 + nd, :],
                    )
                    out_off += nd * P * csz

                ot = oio.tile([P, DT, csz], f32, tag="ot", name="ot")
                if n < NT - 1:
                    pso = [
                        ps2.tile([P, csz], f32, tag=f"ps2_{d}", name=f"ps2_{d}")
                        for d in range(DT)
                    ]
                    for m in range(MT):
                        for d in range(DT):
                            nc.tensor.matmul(
                                pso[d],
                                lhsT=w2_t[m // 4][:, m % 4, d * P : (d + 1) * P],
                                rhs=hT[:, m, :],
                                start=(m == 0),
                                stop=(m == MT - 1),
                            )
                    for d in range(DT):
                        evac(pso[d], d, ot)
                    store(ot, 0, DT)  # one trigger per n-tile
                else:
                    # last tile: d-outer so each d's evacuation + store
                    # overlaps the remaining matmuls (shorter tail)
                    for d in range(DT):
                        pso_d = ps2.tile(
                            [P, csz], f32, tag=f"ps2_{d}", name=f"ps2_{d}"
                        )
                        for m in range(MT):
                            nc.tensor.matmul(
                                pso_d,
                                lhsT=w2_t[m // 4][:, m % 4, d * P : (d + 1) * P],
                                rhs=hT[:, m, :],
                                start=(m == 0),
                                stop=(m == MT - 1),
                            )
                        evac(pso_d, d, ot)
                        store(ot, d, 1)

    nc.compile()
    return nc


def _run(nc, in_maps, trace=False):
    from concourse.bass_utils import run_bass_kernel_spmd

    if trace:
        # register the NTFF profiling hook (missing antenv.axon_hooks shim)
        import types

        import antenv

        if not hasattr(antenv, "axon_hooks"):
            mod = types.ModuleType("antenv.axon_hooks")
            _hook = [None]
            mod.set_axon_ntff_profile_hook = lambda h: _hook.__setitem__(0, h)
            mod.get_axon_ntff_profile_hook = lambda: _hook[0]
            sys.modules["antenv.axon_hooks"] = mod
            antenv.axon_hooks = mod
            from trn_agent_boot.trn_boot import _ntff_profile_via_ctypes

            mod.set_axon_ntff_profile_hook(
                _ntff_profile_via_ctypes("/opt/axon/libaxon_pjrt.so")
            )
    return run_bass_kernel_spmd(
        nc, in_maps, core_ids=list(range(N_CORES)), trace=trace
    )


def kernel(x, gate_w, gate_b, w1, b1, w2, b2, _trace=False):
    x = np.ascontiguousarray(np.asarray(x, dtype=np.float32))
    gate_w = np.asarray(gate_w, dtype=np.float32)
    gate_b = np.asarray(gate_b, dtype=np.float32)
    w1 = np.asarray(w1, dtype=np.float32)
    b1 = np.asarray(b1, dtype=np.float32)
    w2 = np.asarray(w2, dtype=np.float32)
    b2 = np.asarray(b2, dtype=np.float32)

    B, S, D = x.shape
    E = gate_w.shape[1]
    H = w1.shape[2]
    assert E == N_CORES
    T = B * S
    x_flat = x.reshape(T, D)

    top_w, top_idx = _route(x_flat, gate_w, gate_b)

    toks, gvals = [], []
    for e in range(E):
        mask = top_idx == e  # [T, K]; at most one True per row
        t_ids = np.nonzero(mask.any(axis=1))[0]
        toks.append(t_ids)
        gvals.append(top_w[mask].astype(np.float32))
    Cmax = max(len(t) for t in toks)
    C = max(((Cmax + P - 1) // P) * P, NTILE)

    in_maps = []
    for e in range(E):
        cnt = len(toks[e])
        XT = np.zeros((D, C), np.float32)
        XT[:, :cnt] = x_flat[toks[e]].T
        G = np.zeros((1, C), np.float32)
        G[0, :cnt] = gvals[e]
        in_maps.append(_pack_inputs(XT, G, w1[e], b1[e], w2[e], b2[e], C, D, H))

    nc = _build_program(C, D, H, MM_DT)
    res = _run(nc, in_maps, trace=_trace)
    global _LAST_RES
    _LAST_RES = res

    out_flat = np.zeros((T, D), np.float32)
    for e in range(E):
        cnt = len(toks[e])
        outT = _unpack_out(res.results[e]["out"], C, D)
        out_flat[toks[e]] += outT[:, :cnt].T

    out = out_flat.reshape(B, S, D)
    if _trace:
        return out, res.exec_time_ns
    return out
